# revision 29
# baseline (speedup 1.0000x reference)
"""Trainium2 Bass kernel for nn_Enhancement_77309412162.

Math reduction (from the reference):
  theta[b,n] = sum_c x[b,c,n]*theta_w[c] + theta_b        (per-sample matvec)
  g[b,n]     = sum_c x[b,c,n]*g_w[c] + g_b
  phi1[b,n]  = sum_c x1[b,c,n]*phi_w[c] + phi_b
  phi2[b,n]  = sum_c x2[b,c,n]*phi_w[c] + phi_b
  The (N,N) affinity matrices are rank-1, so
  y[b,n] = s_b * theta[b,n],  s_b = (b/N)*(a_c*<phi1,g> + (1-a_c)*<phi2,g>)
  wy[b,c,n] = W_w[c]*t[b,n] + W_b[c],  t = s_b*theta_b
  BN over (B,H,W):  mean[c] = W_w[c]*mu + W_b[c],  var[c] = W_w[c]^2*var_t
  where mu/var_t are the global scalar mean/var of t over all (b,n).
  out[b,c,n] = x[b,c,n] + alpha[c]*(t[b,n]-mu) + bn_b[c]
  with alpha[c] = bn_w[c]*W_w[c]/sqrt(W_w[c]^2*var_t + 1e-5).

Sharding: batch-parallel, one sample per core (B=8, 8 cores). The only
cross-core data is an AllGather of [sum(t), sum(t^2)] (8 bytes/core).

Implementation notes (v2):
- All 12 input tiles (x, x1, x2 channel-chunks) are enqueued on the two
  HWDGE rings up front and stay resident; nothing data-dependent is
  queued between them, so the rings stream at full rate.
- All three projections run on PE in fp32r (1 cyc/col vs 4 for fp32);
  phi1 is projected like phi2 (no broadcast g needed), and the
  <phi,g> dots are chunked DVE mul+reduce on the (2,N) rows.
- theta is broadcast to 128 partitions with K=1 ones-matmuls into PSUM
  (copied out on Pool/DVE), not a DRAM bounce: no extra HBM traffic.
- A dummy AllGather issued as the first Pool-engine instruction absorbs
  the collective firmware boot + barrier under the input stream; the
  real 8-byte AllGather's in/out staging runs on HWDGE (low latency).
- Tail: per channel-chunk ACT(scale*theta_bc+bias) then residual adds
  split DVE/Pool per half-row, each half stored immediately.
"""

import os
import numpy as np

B, C, H, W = 8, 512, 48, 48
N = H * W            # 2304
P = 128
J = C // P           # 4 channel chunks
NCHUNKS = [(0, 512), (512, 512), (1024, 512), (1536, 512), (2048, 256)]
NC5 = len(NCHUNKS)
NCORES = 8
BN_COUNT = float(B * N)

_cache = {}


def _rdma_butterfly(tc, pool, upair, rsems, f32):
    """Allreduce-add of the (128,2) `upair` across the 8 cores of the chip
    via 3 rounds of XOR-partner exchange over direct SBUF->SBUF remote DMA.

    Round r: send my running sum to tpb (mine XOR 2^r) with a relative-dest
    broadcast descriptor, wait for my partner's value (remote sem +2: two
    DMA lanes per dest at n_dests=8), add. All on GpSimd so program order
    is execution order inside the critical section.
    """
    nc = tc.nc
    P = 128
    ucur = upair
    rts = [pool.tile([P, 2], f32, name=f"rt{r}") for r in range(3)]
    uns = [pool.tile([P, 2], f32, name=f"un{r}") for r in range(3)]
    lsem, asem = rsems[3], rsems[4]
    with tc.tile_critical():
        for r in range(3):
            delta = 1 << r
            rdests = [None] * 8
            rdests[delta] = (0, delta)
            if r > 0:
                nc.gpsimd.wait_ge(asem, r)
            nc.gpsimd.remote_dma_broadcast(
                out_ap=rts[r][:], in_ap=ucur[:],
                remote_sem=rsems[r], local_sem=lsem,
                rdests=rdests)
            nc.gpsimd.trigger_dma(1)
            nc.gpsimd.wait_ge(rsems[r], 2)
            nc.gpsimd.tensor_add(uns[r], ucur, rts[r]).then_inc(asem, 1)
            ucur = uns[r]
    return ucur


def _build_nc():
    import concourse.bass as bass
    import concourse.bacc as bacc
    import concourse.tile as tile
    from concourse import mybir
    from contextlib import ExitStack

    f32 = mybir.dt.float32
    f32r = mybir.dt.float32r
    use_f32r = int(os.environ.get("KERNEL_F32R", "1"))
    mm_dt = f32r if use_f32r else f32

    def asf32(ap):
        # f32 view of an mm_dt tile for non-matmul consumers
        return ap.bitcast(f32) if use_f32r else ap

    Alu = mybir.AluOpType
    Act = mybir.ActivationFunctionType
    AxX = mybir.AxisListType.X

    nc = bacc.Bacc("TRN2", target_bir_lowering=False, debug=False,
                   enable_asserts=False, num_devices=NCORES)

    x_d = nc.dram_tensor("x", [C, N], f32, kind="ExternalInput").ap()
    x1_d = nc.dram_tensor("x1", [C, N], f32, kind="ExternalInput").ap()
    x2_d = nc.dram_tensor("x2", [C, N], f32, kind="ExternalInput").ap()
    thw_d = nc.dram_tensor("theta_w", [C], f32, kind="ExternalInput").ap()
    gw_d = nc.dram_tensor("g_w", [C], f32, kind="ExternalInput").ap()
    phw_d = nc.dram_tensor("phi_w", [C], f32, kind="ExternalInput").ap()
    thb_d = nc.dram_tensor("theta_b", [1], f32, kind="ExternalInput").ap()
    gb_d = nc.dram_tensor("g_b", [1], f32, kind="ExternalInput").ap()
    phb_d = nc.dram_tensor("phi_b", [1], f32, kind="ExternalInput").ap()
    ww_d = nc.dram_tensor("W_w", [C], f32, kind="ExternalInput").ap()
    bnw_d = nc.dram_tensor("bn_w", [C], f32, kind="ExternalInput").ap()
    bnb_d = nc.dram_tensor("bn_b", [C], f32, kind="ExternalInput").ap()
    a_d = nc.dram_tensor("a", [1], f32, kind="ExternalInput").ap()
    b_d = nc.dram_tensor("b", [1], f32, kind="ExternalInput").ap()
    out_d = nc.dram_tensor("out", [C, N], f32, kind="ExternalOutput").ap()

    with tile.TileContext(nc) as tc, ExitStack() as ctx:
        singles = ctx.enter_context(tc.tile_pool(name="singles", bufs=1))
        xpool = ctx.enter_context(tc.tile_pool(name="xpool", bufs=3 * J))
        tmps = ctx.enter_context(tc.tile_pool(name="tmps", bufs=3))
        scr = ctx.enter_context(tc.tile_pool(name="scr", bufs=2))
        psproj = ctx.enter_context(tc.tile_pool(name="psproj", bufs=3, space="PSUM"))
        psbc = ctx.enter_context(tc.tile_pool(name="psbc", bufs=2, space="PSUM"))
        psr = ctx.enter_context(tc.tile_pool(name="psr", bufs=1, space="PSUM"))
        dram = ctx.enter_context(tc.tile_pool(name="dram", bufs=1, space="DRAM"))

        cc_mode = os.environ.get("KERNEL_CC", "ncfw")
        rsems = None
        if cc_mode == "rdma":
            # semaphores for the remote-DMA butterfly; same program on every
            # core => same indices everywhere. Cleared here, long before any
            # peer's butterfly sends can land (hardware does not zero
            # semaphores between runs).
            rsems = [nc.alloc_semaphore(name=f"rdma_sem{i}") for i in range(5)]
            for s in rsems[:3] + rsems[4:]:
                nc.gpsimd.sem_clear(s)

        # ---- warm-up collective: the very first Pool-engine instruction,
        # so the cc firmware boot + device barrier overlap the input stream
        if int(os.environ.get("KERNEL_CC_WARM", "1")):
            warm_in = dram.tile([1, 2], f32, name="warm_in")
            warm_out = dram.tile([1, 2 * NCORES], f32, name="warm_out")
            nc.gpsimd.collective_compute(
                "AllGather", Alu.bypass,
                replica_groups=[list(range(NCORES))],
                ins=[warm_in.opt()], outs=[warm_out.opt()],
            )

        # ---- bulk input loads: all 12 tiles enqueued up front on the two
        # HWDGE rings, nothing queued between them, everything resident ----
        _dge = [nc.sync, nc.scalar]
        _dge_i = [0]

        def bulk_dma(out, in_):
            eng = _dge[_dge_i[0] % 2]
            _dge_i[0] += 1
            eng.dma_start(out=out, in_=in_)

        # ---- constant loads FIRST on the HWDGE rings, row-contiguous (one
        # 2KB packet each, vs 512 partition-scatter packets for a direct
        # (j p)->p j load); tiny PE identity-matmuls redistribute them to
        # the [128, J, k] layouts (psum[i, q] = flat[q, j*128+i])
        wxf = singles.tile([2, C], f32, name="wxf")      # [theta_w; g_w]
        bulk_dma(wxf[0:1, :], thw_d[None, :])
        bulk_dma(wxf[1:2, :], gw_d[None, :])
        wpf = singles.tile([2, C], f32, name="wpf")      # [phi_w; phi_w]
        bulk_dma(wpf[0:1, :], phw_d[None, :])
        bulk_dma(wpf[1:2, :], phw_d[None, :])
        wcf = singles.tile([3, C], f32, name="wcf")      # [W_w; bn_w; bn_b]
        bulk_dma(wcf[0:1, :], ww_d[None, :])
        bulk_dma(wcf[1:2, :], bnw_d[None, :])
        bulk_dma(wcf[2:3, :], bnb_d[None, :])

        thgb = singles.tile([2, 1], f32, name="thgb")   # row0 theta_b, row1 g_b
        bulk_dma(thgb[0:1, :], thb_d[None, :])
        bulk_dma(thgb[1:2, :], gb_d[None, :])
        phb2 = singles.tile([2, 1], f32, name="phb2")
        bulk_dma(phb2, bass.AP(tensor=phb_d.tensor, offset=phb_d.offset,
                               ap=[[0, 2], [1, 1]]))

        # identity built with tiny SBUF->SBUF DMAs (engine ops can't
        # address partitions off the 32-aligned bases; DMA can)
        i3 = singles.tile([3, 3], f32, name="i3")
        nc.vector.memset(i3, 0.0)
        one11 = singles.tile([1, 1], f32, name="one11")
        nc.vector.memset(one11, 1.0)
        for q in range(3):
            bulk_dma(i3[q:q + 1, q:q + 1], one11)

        wxt = singles.tile([P, J, 2], mm_dt, name="wxt")     # [theta_w | g_w]
        wpt = singles.tile([P, J, 2], mm_dt, name="wpt")     # [phi_w | phi_w]
        wcb = singles.tile([P, J, 3], f32, name="wcb")       # [W_w|bn_w|bn_b]
        for j in range(J):
            cs = slice(j * P, (j + 1) * P)
            for flat, dst, k in ((wxf, wxt, 2), (wpf, wpt, 2), (wcf, wcb, 3)):
                pw = psbc.tile([P, 4], f32, name="pw")
                nc.tensor.matmul(pw[:, :k], lhsT=flat[:, cs], rhs=i3[:k, :k],
                                 start=True, stop=True)
                nc.scalar.activation(out=dst[:, j, :], in_=pw[:, :k],
                                     func=Act.Copy)
        ww = wcb[:, :, 0]
        bnw = wcb[:, :, 1]
        bnb = wcb[:, :, 2]

        # input tiles carry mm_dt so the DMA itself "rounds" them for the
        # fp32r matmuls (bit-identical); f32 consumers use a bitcast view
        x_tiles, x1_tiles, x2_tiles = [], [], []
        for src_d, tiles in ((x_d, x_tiles), (x1_d, x1_tiles), (x2_d, x2_tiles)):
            for j in range(J):
                xt = xpool.tile([P, N], mm_dt, name="xt")
                src = src_d[j * P:(j + 1) * P, :]
                bulk_dma(xt, src.bitcast(mm_dt) if use_f32r else src)
                tiles.append(xt)

        ones1 = singles.tile([1, P], mm_dt, name="ones1")
        if use_f32r:
            ones1f = singles.tile([1, P], f32, name="ones1f")
            nc.vector.memset(ones1f, 1.0)
            nc.scalar.activation(out=ones1, in_=ones1f, func=Act.Copy)
        else:
            nc.vector.memset(ones1, 1.0)
        # selector lhsTs: sel0 broadcasts partition-0 values to all 128
        # partitions, sel1 broadcasts partition-1 values
        sel0 = singles.tile([2, P], f32, name="sel0")
        nc.vector.memset(sel0, 0.0)
        nc.vector.memset(sel0[0:1, :], 1.0)
        sel1 = singles.tile([2, P], f32, name="sel1")
        nc.vector.memset(sel1, 1.0)
        nc.vector.tensor_sub(sel1, sel1, sel0)
        # partials: col0 row0=A=sum(theta), col1 row0=B=sum(theta^2),
        # col2 row1=d2=<phi2,g>, col3 row1=d1=<phi1,g> (biases included),
        # col4 row0=a, col5 row0=b (scalars ride the broadcast matmul)
        PT = singles.tile([2, 6], f32, name="PT")
        bulk_dma(PT[0:1, 4:5], a_d[None, :])
        bulk_dma(PT[0:1, 5:6], b_d[None, :])

        # thg carries mm_dt (ACT "rounds" on write) so the broadcast
        # matmul can consume theta directly; f32 consumers bitcast
        thg = singles.tile([2, N], mm_dt, name="thg")   # row0 theta, row1 g

        def project(dst, w_pj, b_21, src_tiles):
            # dst[{0,1}, n] = sum_c src[c, n] * w[c, {0,1}] + b
            for (n0, nsz) in NCHUNKS:
                ps = psproj.tile([2, 512], f32, name="ps_proj")
                for j in range(J):
                    nc.tensor.matmul(ps[:, :nsz], lhsT=w_pj[:, j, :],
                                     rhs=src_tiles[j][:, n0:n0 + nsz],
                                     start=(j == 0), stop=(j == J - 1))
                nc.scalar.activation(out=dst[:, n0:n0 + nsz], in_=ps[:, :nsz],
                                     func=Act.Identity, bias=b_21, scale=1.0)

        project(thg, wxt, thgb, x_tiles)

        # theta broadcast to all 128 partitions via K=1 ones-matmuls into
        # PSUM (PE is otherwise idle; no HBM traffic), copies on Pool/DVE
        theta_bc = singles.tile([P, N], f32, name="theta_bc")
        for k, (n0, nsz) in enumerate(NCHUNKS):
            pb = psbc.tile([P, 512], f32, name="pb")
            nc.tensor.matmul(pb[:, :nsz], lhsT=ones1,
                             rhs=thg[0:1, n0:n0 + nsz],
                             start=True, stop=True)
            if k % 2 == 0:
                nc.vector.tensor_copy(theta_bc[:, n0:n0 + nsz], pb[:, :nsz])
            else:
                nc.scalar.activation(out=theta_bc[:, n0:n0 + nsz],
                                     in_=pb[:, :nsz], func=Act.Copy)

        # A = sum(theta) (row0 of col0), B = sum(theta^2) (row0 of col1)
        sq_scr = scr.tile([2, N], f32, name="sq_scr")
        nc.scalar.activation(out=sq_scr, in_=asf32(thg[:, :]), func=Act.Identity,
                             accum_out=PT[:, 0:1])
        nc.scalar.activation(out=sq_scr, in_=asf32(thg[:, :]), func=Act.Square,
                             accum_out=PT[:, 1:2])

        # phi projections (duplicated weights put phi on both rows), then
        # chunked mul+reduce against thg gives <phi,g> on row 1
        def phi_dot(src_tiles, pt_col, nm):
            phi = singles.tile([2, N], f32, name=nm)
            project(phi, wpt, phb2, src_tiles)
            dk = singles.tile([2, NC5], f32, name=nm + "k")
            for k, (n0, nsz) in enumerate(NCHUNKS):
                d_scr = scr.tile([2, 512], f32, name="d_scr")
                nc.vector.tensor_mul(d_scr[:, :nsz], phi[:, n0:n0 + nsz],
                                     asf32(thg[:, n0:n0 + nsz]))
                nc.vector.tensor_reduce(dk[:, k:k + 1], d_scr[:, :nsz],
                                        axis=AxX, op=Alu.add)
            nc.vector.tensor_reduce(PT[:, pt_col:pt_col + 1], dk,
                                    axis=AxX, op=Alu.add)

        phi_dot(x1_tiles, 3, "phi1")
        phi_dot(x2_tiles, 2, "phi2")

        # broadcast the (2,6) partials rows to all 128 partitions
        pr = psr.tile([P, 2 * NCORES], f32, name="prx")
        nc.tensor.matmul(pr[:, 0:6], lhsT=sel0, rhs=PT, start=True, stop=True)
        nc.tensor.matmul(pr[:, 6:12], lhsT=sel1, rhs=PT, start=True, stop=True)
        r_sb = singles.tile([P, 12], f32, name="r_sb")
        nc.scalar.activation(out=r_sb, in_=pr[:, 0:12], func=Act.Copy)
        A_ = r_sb[:, 0:1]
        B_ = r_sb[:, 1:2]
        a_ = r_sb[:, 4:5]
        b_ = r_sb[:, 5:6]
        d2_ = r_sb[:, 8:9]
        d1_ = r_sb[:, 9:10]

        # s = (b/N) * (d2 + a_c*(d1-d2)); u1 = s*A; u2 = s^2*B  (replicated)
        ac = singles.tile([P, 1], f32, name="ac")
        nc.vector.tensor_scalar(ac, a_, 0.0, 1.0, op0=Alu.max, op1=Alu.min)
        sv = singles.tile([P, 1], f32, name="sv")
        nc.vector.tensor_sub(sv, d1_, d2_)
        nc.vector.tensor_mul(sv, sv, ac)
        nc.vector.tensor_add(sv, sv, d2_)
        nc.vector.tensor_mul(sv, sv, b_)
        nc.vector.tensor_scalar_mul(sv, sv, 1.0 / float(N))
        s2v = singles.tile([P, 1], f32, name="s2v")
        nc.vector.tensor_mul(s2v, sv, sv)
        upair = singles.tile([P, 2], f32, name="upair")
        nc.vector.tensor_mul(upair[:, 0:1], sv, A_)
        nc.vector.tensor_mul(upair[:, 1:2], s2v, B_)

        # ---- 8-byte-per-core allreduce across the 8 cores ----
        if cc_mode == "rdma":
            # 3-round XOR-butterfly over direct SBUF->SBUF remote DMA
            uu = _rdma_butterfly(tc, singles, upair, rsems, f32)
        else:
            # AllGather via ncfw; staging on HWDGE for low latency
            cc_in = dram.tile([1, 2], f32, name="cc_in")
            cc_out = dram.tile([1, 2 * NCORES], f32, name="cc_out")
            nc.sync.dma_start(out=cc_in, in_=upair[0:1, :])
            nc.gpsimd.collective_compute(
                "AllGather", Alu.bypass,
                replica_groups=[list(range(NCORES))],
                ins=[cc_in.opt()], outs=[cc_out.opt()],
            )
            # single-packet readback to one partition, PE ones-matmul
            # broadcast to 128 (vs a 128-packet stride-0 DMA), reduce
            # straight out of PSUM
            bc1 = singles.tile([1, 2 * NCORES], mm_dt, name="bc1")
            nc.sync.dma_start(
                out=bc1,
                in_=cc_out[:, :].bitcast(mm_dt) if use_f32r else cc_out[:, :])
            pbg = psr.tile([P, 2 * NCORES], f32, name="prx")
            nc.tensor.matmul(pbg, lhsT=ones1, rhs=bc1, start=True, stop=True)
            uu = singles.tile([P, 2], f32, name="uu")
            nc.vector.tensor_reduce(uu, pbg.rearrange("p (r i) -> p i r", i=2),
                                    axis=AxX, op=Alu.add)

        # global stats -> per-channel scale/bias (column j = channels j*128+p)
        muv = singles.tile([P, 1], f32, name="muv")
        nc.vector.tensor_scalar_mul(muv, uu[:, 0:1], 1.0 / BN_COUNT)
        varv = singles.tile([P, 1], f32, name="varv")
        nc.vector.tensor_scalar_mul(varv, uu[:, 1:2], 1.0 / BN_COUNT)
        musq = singles.tile([P, 1], f32, name="musq")
        nc.vector.tensor_mul(musq, muv, muv)
        nc.vector.tensor_sub(varv, varv, musq)
        dv = singles.tile([P, J], f32, name="dv")
        nc.vector.tensor_mul(dv, ww, ww)
        nc.vector.tensor_scalar(dv, dv, varv, 1e-5, op0=Alu.mult, op1=Alu.add)
        nc.scalar.activation(out=dv, in_=dv, func=Act.Sqrt)
        rst = singles.tile([P, J], f32, name="rst")
        nc.vector.reciprocal(rst, dv)
        alpha = singles.tile([P, J], f32, name="alpha")
        nc.vector.tensor_mul(alpha, bnw, ww)
        nc.vector.tensor_mul(alpha, alpha, rst)
        scale2 = singles.tile([P, J], f32, name="scale2")
        nc.vector.tensor_scalar(scale2, alpha, sv, None, op0=Alu.mult)
        bias2 = singles.tile([P, J], f32, name="bias2")
        nc.vector.tensor_scalar(bias2, alpha, muv, None, op0=Alu.mult)
        nc.vector.tensor_sub(bias2, bnb, bias2)

        # out = x + scale2[c]*theta_bc + bias2[c], in half-row units.
        # All 8 scale/bias ACT halves issue FIRST so the ScalarE queue is
        # never blocked behind a scalar-ring store's semaphore wait (the
        # ring issue runs on ScalarE). Pool (3.7us/half) takes only the 2
        # earliest halves, DVE (1.35us/half) the rest. The scalar ring
        # only gets early-completing stores; the sync ring (idle Sync
        # engine) carries the others.
        HALF = N // 2
        tmp_tiles = []
        for j in range(J):
            tmp = tmps.tile([P, N], f32, name="tmp")
            tmp_tiles.append(tmp)
            for h in range(2):
                sl = slice(h * HALF, (h + 1) * HALF)
                nc.scalar.activation(out=tmp[:, sl], in_=theta_bc[:, sl],
                                     func=Act.Identity,
                                     scale=scale2[:, j:j + 1],
                                     bias=bias2[:, j:j + 1])
        pool_halves = {(0, 1), (1, 1)}
        scalar_ring = {(0, 0), (0, 1), (1, 0)}
        for j in range(J):
            for h in range(2):
                sl = slice(h * HALF, (h + 1) * HALF)
                eng = nc.gpsimd if (j, h) in pool_halves else nc.vector
                eng.tensor_add(x_tiles[j][:, sl],
                               asf32(x_tiles[j][:, sl]), tmp_tiles[j][:, sl])
                seng = nc.scalar if (j, h) in scalar_ring else nc.sync
                seng.dma_start(out=out_d[j * P:(j + 1) * P, sl],
                               in_=asf32(x_tiles[j][:, sl]))

    nc.compile()
    return nc


def kernel(**inputs):
    from concourse import bass_utils

    nc = _cache.get("nc")
    if nc is None:
        nc = _build_nc()
        _cache["nc"] = nc

    def f32c(a):
        return np.ascontiguousarray(np.asarray(a, dtype=np.float32))

    xs = f32c(inputs["x"]).reshape(B, C, N)
    x1s = f32c(inputs["x1"]).reshape(B, C, N)
    x2s = f32c(inputs["x2"]).reshape(B, C, N)
    shared = {
        "theta_w": f32c(inputs["theta_w"]),
        "g_w": f32c(inputs["g_w"]),
        "phi_w": f32c(inputs["phi_w"]),
        "theta_b": f32c(inputs["theta_b"]),
        "g_b": f32c(inputs["g_b"]),
        "phi_b": f32c(inputs["phi_b"]),
        "W_w": f32c(inputs["W_w"]),
        "bn_w": f32c(inputs["bn_w"]),
        "bn_b": f32c(inputs["bn_b"]),
        "a": f32c(inputs["a"]),
        "b": f32c(inputs["b"]),
    }
    in_maps = [
        {"x": xs[c], "x1": x1s[c], "x2": x2s[c], **shared}
        for c in range(NCORES)
    ]
    res = bass_utils.run_bass_kernel_spmd(
        nc, in_maps, core_ids=list(range(NCORES)),
        trace=bool(os.environ.get("BASS_TRACE")),
        tmpdir=os.environ.get("KERNEL_TMPDIR") or None,
    )
    _cache["last_results"] = res
    out = np.stack([res.results[c]["out"] for c in range(NCORES)], axis=0)
    return out.reshape(B, C, H, W)
